# revision 1
# baseline (speedup 1.0000x reference)
"""MoE (top-1 routing, capacity-dropped) forward on 8 Trainium2 NeuronCores.

Strategy (expert-parallel, per the sharding hint):
  - Host computes the top-1 gating (softmax over E=8 logits per token), the
    per-expert token ranks (cumsum order), capacity dropping, and dispatches
    token rows to their expert: this *is* the sharding step — tokens are
    sharded along the expert axis E, one expert per NeuronCore, with W1/b1/
    W2/b2 sharded along E and the gate Wg applied once on the (replicated)
    full token set.
  - Each core runs the heavy expert FFN on its C=1024 dispatched tokens in
    transposed [feature, token] layout so both matmuls consume the natural
    weight layouts:  yT = W2[e].T @ relu(W1[e].T @ xT + b1[e]) + b2[e].
    Matmuls run as float32r (full fp32 data, full-rate PE mode).
  - Host combine: scatter each expert's output rows back to their token
    positions scaled by the gate value (zero rows for dropped tokens).

Hardcoded shapes: x [4, 2048, 1024], Wg [1024, 8], W1 [8, 1024, 4096],
b1 [8, 4096], W2 [8, 4096, 1024], b2 [8, 1024].
"""

import os
import numpy as np
from contextlib import ExitStack

# The axon NTFF profiling hook is absent in some client environments; if
# tracing is requested via BASS_TRACE but the hook can't be imported,
# run_bass_kernel_spmd would crash. Disable tracing only in that case.
try:
    from antenv.axon_hooks import get_axon_ntff_profile_hook  # noqa: F401
except Exception:
    os.environ.setdefault("BASS_NEVER_TRACE", "1")

from concourse import bacc, mybir, tile
from concourse.bass_utils import run_bass_kernel_spmd

B, S, H, F, E = 4, 2048, 1024, 4096, 8
T = B * S                  # 8192 tokens
C = -(-T // E)             # 1024 capacity per expert
P = 128                    # SBUF partitions
NB = 512                   # matmul moving free-dim block (one PSUM bank, fp32)
NCORES = 8

_DT = mybir.dt.float32r    # fp32 bits, full-rate PE matmul mode


def _build_expert_ffn(repeat: int = 1):
    """Per-core program: yT = W2.T @ relu(W1.T @ xT + b1) + b2, all [feat, tok].

    repeat>1 wraps the whole body in a hardware For loop — used only by the
    timing harness to measure steady-state per-iteration HW time via the
    wall-clock slope between repeat counts."""
    nc = bacc.Bacc("TRN2", target_bir_lowering=False, debug=False,
                   num_devices=NCORES)
    KC = 8  # k-tiles per streamed W2 chunk (matches the W1 slab size)
    # Weights arrive pre-tiled from the host (see kernel()) so every slab DMA
    # is a fully contiguous 4KB-per-partition read instead of a strided
    # 512B-descriptor gather: W1p[m, p, k, f] = W1[k*P+p, m*P+f] and
    # W2p[mh, kc, p, k, f] = W2[kc*KC*P + k*P + p, mh*P + f].
    xT = nc.declare_dram_parameter("xT", [H, C], _DT, isOutput=False)
    W1 = nc.declare_dram_parameter("W1", [F // P, P, H // P, P], _DT, isOutput=False)
    W2 = nc.declare_dram_parameter(
        "W2", [H // P, F // (KC * P), P, KC, P], _DT, isOutput=False)
    b1 = nc.declare_dram_parameter("b1", [P, F // P], mybir.dt.float32,
                                   isOutput=False)  # host-pretiled [p, m]
    b2 = nc.declare_dram_parameter("b2", [P, H // P], mybir.dt.float32,
                                   isOutput=False)
    out = nc.declare_dram_parameter("out", [H, C], mybir.dt.float32, isOutput=True)

    KH = H // P    # 8  k-tiles over H (mm1 contraction)
    KF = F // P    # 32 k-tiles over F (mm2 contraction)
    MF = F // P    # 32 m-tiles over F (mm1 output partitions)
    MH = H // P    # 8  m-tiles over H (mm2 output partitions)
    NBLK = C // NB  # 2 moving blocks over the C tokens

    with tile.TileContext(nc) as tc, ExitStack() as ctx:
        xpool = ctx.enter_context(tc.tile_pool(name="xpool", bufs=1))
        h1pool = ctx.enter_context(tc.tile_pool(name="h1pool", bufs=1))
        cpool = ctx.enter_context(tc.tile_pool(name="cpool", bufs=1))
        wpool = ctx.enter_context(tc.tile_pool(name="wpool", bufs=6))
        ypool = ctx.enter_context(tc.tile_pool(name="ypool", bufs=2))
        psum = ctx.enter_context(tc.tile_pool(name="psum", bufs=6, space="PSUM"))

        loop_ctx = tc.For_i(0, repeat, 1) if repeat > 1 else None
        if loop_ctx is not None:
            loop_ctx.__enter__()

        # biases arrive host-pretiled: b1[p, m] = b1_orig[m*P + p]
        b1_sb = cpool.tile([P, MF], mybir.dt.float32, name="b1_sb")
        nc.sync.dma_start(out=b1_sb[:], in_=b1[:])
        b2_sb = cpool.tile([P, MH], mybir.dt.float32, name="b2_sb")
        nc.sync.dma_start(out=b2_sb[:], in_=b2[:])

        # resident activations
        x_sb = xpool.tile([P, KH, C], _DT, name="x_sb")
        for k in range(KH):
            nc.sync.dma_start(out=x_sb[:, k, :], in_=xT[k * P:(k + 1) * P, :])
        h1_sb = h1pool.tile([P, KF, C], _DT, name="h1_sb")

        # mm1: h1[m*P+p, c] = relu(b1[m*P+p] + sum_h W1[h, m*P+p] * xT[h, c])
        for m in range(MF):
            w1s = wpool.tile([P, KH, P], _DT, tag="wslab", name="w1s")
            nc.scalar.dma_start(out=w1s[:], in_=W1[m])
            for n in range(NBLK):
                ps = psum.tile([P, NB], mybir.dt.float32, tag="ps", name="ps")
                for k in range(KH):
                    nc.tensor.matmul(
                        out=ps[:],
                        lhsT=w1s[:, k, :],
                        rhs=x_sb[:, k, n * NB:(n + 1) * NB],
                        start=(k == 0),
                        stop=(k == KH - 1),
                    )
                nc.scalar.activation(
                    out=h1_sb[:, m, n * NB:(n + 1) * NB],
                    in_=ps[:],
                    func=mybir.ActivationFunctionType.Relu,
                    bias=b1_sb[:, m:m + 1],
                )

        # mm2: y[mh*P+p, c] = b2[mh*P+p] + sum_f W2[f, mh*P+p] * h1[f, c]
        for mh in range(MH):
            w2chunks = []
            for kc in range(KF // KC):
                w2s = wpool.tile([P, KC, P], _DT, tag="wslab", name="w2s")
                nc.scalar.dma_start(out=w2s[:], in_=W2[mh, kc])
                w2chunks.append(w2s)
            for n in range(NBLK):
                ps2 = psum.tile([P, NB], mybir.dt.float32, tag="ps", name="ps2")
                for k in range(KF):
                    nc.tensor.matmul(
                        out=ps2[:],
                        lhsT=w2chunks[k // KC][:, k % KC, :],
                        rhs=h1_sb[:, k, n * NB:(n + 1) * NB],
                        start=(k == 0),
                        stop=(k == KF - 1),
                    )
                y_sb = ypool.tile([P, NB], mybir.dt.float32, tag="y", name="y_sb")
                nc.vector.tensor_add(
                    out=y_sb[:],
                    in0=ps2[:],
                    in1=b2_sb[:, mh:mh + 1].to_broadcast([P, NB]),
                )
                nc.gpsimd.dma_start(
                    out=out[mh * P:(mh + 1) * P, n * NB:(n + 1) * NB], in_=y_sb[:]
                )
        if loop_ctx is not None:
            loop_ctx.__exit__(None, None, None)
    nc.compile()
    return nc


_NC_CACHE = None


def _get_nc():
    global _NC_CACHE
    if _NC_CACHE is None:
        _NC_CACHE = _build_expert_ffn()
    return _NC_CACHE


def _route(tokens: np.ndarray, Wg: np.ndarray):
    """Top-1 gating with capacity C, matching deepspeed top1gating semantics.

    Runs on jax-CPU mirroring the reference ops 1:1 so the discrete routing
    decisions (argmax expert, cumsum slot order, capacity drops) are
    bit-identical to the jax reference — a numpy BLAS logits matmul could
    flip the argmax for tokens whose top-2 logits are ~1e-6 apart.

    Returns per-expert kept token ids (slot order) and per-token gate values
    (0 for dropped tokens)."""
    import jax
    import jax.numpy as jnp

    cpu = jax.devices("cpu")[0]
    with jax.default_device(cpu):
        tok = jnp.asarray(tokens)
        logits = tok @ jnp.asarray(Wg)                       # [T, E]
        gates = jax.nn.softmax(logits, axis=-1)
        idx = jnp.argmax(gates, axis=1)
        mask1 = jax.nn.one_hot(idx, E, dtype=gates.dtype)
        locations1 = jnp.cumsum(mask1, axis=0) - 1.0
        mask1 = mask1 * (locations1 < C).astype(gates.dtype)
        gates1 = jnp.sum(gates * mask1, axis=1)              # 0 if dropped

        mask_np = np.asarray(mask1)
        gate_val = np.asarray(gates1, dtype=np.float32)

    tok_ids = [np.nonzero(mask_np[:, e] > 0)[0] for e in range(E)]
    return tok_ids, gate_val


def kernel(x, Wg, W1, b1, W2, b2):
    x = np.asarray(x, dtype=np.float32)
    Wg = np.asarray(Wg, dtype=np.float32)
    W1 = np.asarray(W1, dtype=np.float32)
    b1 = np.asarray(b1, dtype=np.float32)
    W2 = np.asarray(W2, dtype=np.float32)
    b2 = np.asarray(b2, dtype=np.float32)

    tokens = x.reshape(T, H)
    tok_ids, gate_val = _route(tokens, Wg)

    # dispatch: shard tokens along the expert axis, one expert per core.
    # Weights are pre-tiled to the kernel's SBUF slab layout (see
    # _build_expert_ffn) so each slab DMA is contiguous per partition.
    KC = 8
    in_maps = []
    for e in range(E):
        xT_e = np.zeros((H, C), dtype=np.float32)
        ids = tok_ids[e]
        xT_e[:, :len(ids)] = tokens[ids].T
        W1p = np.ascontiguousarray(
            W1[e].reshape(H // P, P, F // P, P).transpose(2, 1, 0, 3))
        W2p = np.ascontiguousarray(
            W2[e].reshape(F // (KC * P), KC, P, H // P, P).transpose(3, 0, 2, 1, 4))
        in_maps.append({
            "xT": xT_e,
            "W1": W1p,
            "W2": W2p,
            "b1": np.ascontiguousarray(b1[e].reshape(F // P, P).T),
            "b2": np.ascontiguousarray(b2[e].reshape(H // P, P).T),
        })

    nc = _get_nc()
    res = run_bass_kernel_spmd(nc, in_maps, list(range(NCORES)))

    # combine: scatter expert outputs back, scaled by the gate value
    out = np.zeros((T, H), dtype=np.float32)
    for e in range(E):
        ids = tok_ids[e]
        yT = res.results[e]["out"]                   # [H, C]
        out[ids] = yT[:, :len(ids)].T * gate_val[ids, None]
    return out.reshape(B, S, H)



# revision 3
# speedup vs baseline: 1.0714x; 1.0714x over previous
"""MoE (top-1 routing, capacity-dropped) forward on 8 Trainium2 NeuronCores.

Expert-parallel (per the sharding hint): host computes top-1 gating +
capacity dropping and dispatches token rows to their expert — tokens sharded
along E (one expert per core), W1/b1/W2/b2 sharded along E, gate Wg applied
once on the full token set. Each core runs the expert FFN on its C=1024
dispatched tokens in [feature, token] layout:
    yT = W2[e].T @ relu(W1[e].T @ xT + b1[e]) + b2[e]
Host combine scatters expert outputs back scaled by the gate value.

The FFN matmuls run in bfloat16 (fp32 PSUM accumulation; biases and the
final output stay fp32). The PE array processes one element per cell per
cycle regardless of width, so bf16 matmul cycle count equals float32r's —
but bf16
  - halves the weight/activation HBM traffic (33.5 MB -> 16.8 MB per core),
  - enables Fast Weight Load (the compiler disables FWL for FP32 weights),
  - halves SBUF pressure (the resident h1 goes 16 MB -> 8 MB),
  - draws less PE/SBUF/DMA power (fp32r full-rate is the worst-case power
    mode; sustained high draw downclocks PE 2.4 -> ~2.0 GHz via P0).
Numerics: rel. Frobenius error 3.3e-3 vs the fp32 reference (fp32r version
was 2.1e-4); the PE-bound floor is 1024 matmuls x 512 cycles = 218.5 us at
the warm 2.4 GHz clock.

Schedule notes: x is host-pretiled into sixteen contiguous 128 KB (n,k)
chunks so the first matmul waits on one chunk, not the whole 2 MB transfer;
W1/W2 stream as 256 KB slabs through a shared 6-deep pool ring (the shared
ring throttles W2 prefetch so it can't starve early x/W1 transfers on the
shared DMA bandwidth); the final output block runs as two 256-col PSUM
groups so the last store overlaps the last matmul group.

Hardcoded shapes: x [4, 2048, 1024], Wg [1024, 8], W1 [8, 1024, 4096],
b1 [8, 4096], W2 [8, 4096, 1024], b2 [8, 1024].
"""

import os
import numpy as np
from contextlib import ExitStack

try:
    from antenv.axon_hooks import get_axon_ntff_profile_hook  # noqa: F401
except Exception:
    os.environ.setdefault("BASS_NEVER_TRACE", "1")

import ml_dtypes
from concourse import bacc, mybir, tile
from concourse.bass_utils import run_bass_kernel_spmd

B, S, H, F, E = 4, 2048, 1024, 4096, 8
T = B * S                  # 8192 tokens
C = -(-T // E)             # 1024 capacity per expert
P = 128                    # SBUF partitions
NB = 512                   # matmul moving free-dim block (one PSUM bank, fp32)
NCORES = 8

_DT = mybir.dt.bfloat16
_NPDT = ml_dtypes.bfloat16


def _build_expert_ffn(repeat: int = 1):
    """Per-core program: yT = W2.T @ relu(W1.T @ xT + b1) + b2, all [feat, tok].

    repeat>1 wraps the whole body in a hardware For loop — used only by the
    timing harness to measure steady-state per-iteration HW time via the
    wall-clock slope between repeat counts."""
    nc = bacc.Bacc("TRN2", target_bir_lowering=False, debug=False,
                   num_devices=NCORES)
    KC = 8  # k-tiles per streamed W2 chunk (matches the W1 slab size)
    # Weights arrive pre-tiled from the host (see kernel()) so every slab DMA
    # is a fully contiguous read per partition:
    # W1p[m, p, k, f] = W1[k*P+p, m*P+f] and
    # W2p[mh, kc, p, k, f] = W2[kc*KC*P + k*P + p, mh*P + f].
    NBLK = C // NB
    # x arrives host-pretiled as (n, k)-chunks, each a fully-contiguous
    # [P, NB] slab (1 KB/partition), so the first matmul waits on one 128 KB
    # chunk instead of the whole 2 MB transfer:
    # xT[n, k, p, c] = x_orig[k*P+p, n*NB+c].
    xT = nc.declare_dram_parameter(
        "xT", [NBLK, H // P, P, NB], _DT, isOutput=False)
    W1 = nc.declare_dram_parameter("W1", [F // P, P, H // P, P], _DT, isOutput=False)
    W2 = nc.declare_dram_parameter(
        "W2", [H // P, F // (KC * P), P, KC, P], _DT, isOutput=False)
    b1 = nc.declare_dram_parameter("b1", [P, F // P], mybir.dt.float32,
                                   isOutput=False)  # host-pretiled [p, m]
    b2 = nc.declare_dram_parameter("b2", [P, H // P], mybir.dt.float32,
                                   isOutput=False)
    out = nc.declare_dram_parameter("out", [H, C], mybir.dt.float32, isOutput=True)

    KH = H // P    # 8  k-tiles over H (mm1 contraction)
    KF = F // P    # 32 k-tiles over F (mm2 contraction)
    MF = F // P    # 32 m-tiles over F (mm1 output partitions)
    MH = H // P    # 8  m-tiles over H (mm2 output partitions)

    with tile.TileContext(nc) as tc, ExitStack() as ctx:
        xpool = ctx.enter_context(tc.tile_pool(name="xpool", bufs=1))
        h1pool = ctx.enter_context(tc.tile_pool(name="h1pool", bufs=1))
        cpool = ctx.enter_context(tc.tile_pool(name="cpool", bufs=1))
        wpool = ctx.enter_context(tc.tile_pool(name="wpool", bufs=6))
        ypool = ctx.enter_context(tc.tile_pool(name="ypool", bufs=2))
        psum = ctx.enter_context(tc.tile_pool(name="psum", bufs=6, space="PSUM"))

        loop_ctx = tc.For_i(0, repeat, 1) if repeat > 1 else None
        if loop_ctx is not None:
            loop_ctx.__enter__()

        # resident activations, chunked (n, k) so compute starts after the
        # first 128 KB chunk; issue order matches consumption order
        x_sb = [[xpool.tile([P, NB], _DT, tag=f"x{n}_{k}", name=f"x_sb{n}_{k}")
                 for k in range(KH)] for n in range(NBLK)]
        for n in range(NBLK):
            for k in range(KH):
                nc.sync.dma_start(out=x_sb[n][k][:], in_=xT[n, k])

        # biases arrive host-pretiled: b1[p, m] = b1_orig[m*P + p]
        b1_sb = cpool.tile([P, MF], mybir.dt.float32, name="b1_sb")
        nc.gpsimd.dma_start(out=b1_sb[:], in_=b1[:])
        b2_sb = cpool.tile([P, MH], mybir.dt.float32, name="b2_sb")
        nc.gpsimd.dma_start(out=b2_sb[:], in_=b2[:])

        h1_sb = h1pool.tile([P, KF, C], _DT, name="h1_sb")

        # mm1: h1[m*P+p, c] = relu(b1[m*P+p] + sum_h W1[h, m*P+p] * xT[h, c])
        for m in range(MF):
            w1s = wpool.tile([P, KH, P], _DT, tag="wslab", name="w1s")
            nc.scalar.dma_start(out=w1s[:], in_=W1[m])
            for n in range(NBLK):
                ps = psum.tile([P, NB], mybir.dt.float32, tag="ps", name="ps")
                for k in range(KH):
                    nc.tensor.matmul(
                        out=ps[:],
                        lhsT=w1s[:, k, :],
                        rhs=x_sb[n][k][:],
                        start=(k == 0),
                        stop=(k == KH - 1),
                    )
                nc.scalar.activation(
                    out=h1_sb[:, m, n * NB:(n + 1) * NB],
                    in_=ps[:],
                    func=mybir.ActivationFunctionType.Relu,
                    bias=b1_sb[:, m:m + 1],
                )

        # mm2: y[mh*P+p, c] = b2[mh*P+p] + sum_f W2[f, mh*P+p] * h1[f, c]
        for mh in range(MH):
            w2chunks = []
            for kc in range(KF // KC):
                w2s = wpool.tile([P, KC, P], _DT, tag="wslab", name="w2s")
                nc.scalar.dma_start(out=w2s[:], in_=W2[mh, kc])
                w2chunks.append(w2s)
            for n in range(NBLK):
                # the very last block runs as two 256-col accumulation groups
                # (same total PE cycles) so its first store overlaps the
                # second group's matmuls — halves the exposed drain tail
                last = (mh == MH - 1 and n == NBLK - 1)
                SB = NB // 2 if last else NB
                for s in range(NB // SB):
                    c0 = n * NB + s * SB
                    ps2 = psum.tile([P, NB], mybir.dt.float32, tag="ps", name="ps2")
                    for k in range(KF):
                        nc.tensor.matmul(
                            out=ps2[:, :SB],
                            lhsT=w2chunks[k // KC][:, k % KC, :],
                            rhs=h1_sb[:, k, c0:c0 + SB],
                            start=(k == 0),
                            stop=(k == KF - 1),
                        )
                    y_sb = ypool.tile([P, NB], mybir.dt.float32, tag="y", name="y_sb")
                    nc.vector.tensor_add(
                        out=y_sb[:, :SB],
                        in0=ps2[:, :SB],
                        in1=b2_sb[:, mh:mh + 1].to_broadcast([P, SB]),
                    )
                    nc.gpsimd.dma_start(
                        out=out[mh * P:(mh + 1) * P, c0:c0 + SB], in_=y_sb[:, :SB]
                    )
        if loop_ctx is not None:
            loop_ctx.__exit__(None, None, None)
    nc.compile()
    return nc


_NC_CACHE = None


def _get_nc():
    global _NC_CACHE
    if _NC_CACHE is None:
        _NC_CACHE = _build_expert_ffn()
    return _NC_CACHE


def _route(tokens: np.ndarray, Wg: np.ndarray):
    """Top-1 gating with capacity C, matching deepspeed top1gating semantics.

    Runs on jax-CPU mirroring the reference ops 1:1 so the discrete routing
    decisions (argmax expert, cumsum slot order, capacity drops) are
    bit-identical to the jax reference — a numpy BLAS logits matmul could
    flip the argmax for tokens whose top-2 logits are ~1e-6 apart.

    Returns per-expert kept token ids (slot order) and per-token gate values
    (0 for dropped tokens)."""
    import jax
    import jax.numpy as jnp

    cpu = jax.devices("cpu")[0]
    with jax.default_device(cpu):
        tok = jnp.asarray(tokens)
        logits = tok @ jnp.asarray(Wg)                       # [T, E]
        gates = jax.nn.softmax(logits, axis=-1)
        idx = jnp.argmax(gates, axis=1)
        mask1 = jax.nn.one_hot(idx, E, dtype=gates.dtype)
        locations1 = jnp.cumsum(mask1, axis=0) - 1.0
        mask1 = mask1 * (locations1 < C).astype(gates.dtype)
        gates1 = jnp.sum(gates * mask1, axis=1)              # 0 if dropped

        mask_np = np.asarray(mask1)
        gate_val = np.asarray(gates1, dtype=np.float32)

    tok_ids = [np.nonzero(mask_np[:, e] > 0)[0] for e in range(E)]
    return tok_ids, gate_val


def _make_in_maps(x, W1, b1, W2, b2, tok_ids):
    """Shard tokens along the expert axis and pre-tile weights to the
    kernel's SBUF slab layout (contiguous per-partition reads), in bf16."""
    KC = 8
    tokens = x.reshape(T, H)
    in_maps = []
    NBLK = C // NB
    for e in range(E):
        xT_e = np.zeros((H, C), dtype=_NPDT)
        ids = tok_ids[e]
        xT_e[:, :len(ids)] = tokens[ids].astype(_NPDT).T
        # kernel SBUF chunk layout: xT[n, k, p, c] = xT_e[k*P+p, n*NB+c]
        xT_t = np.ascontiguousarray(
            xT_e.reshape(H // P, P, NBLK, NB).transpose(2, 0, 1, 3))
        W1p = np.ascontiguousarray(
            W1[e].reshape(H // P, P, F // P, P).transpose(2, 1, 0, 3)
        ).astype(_NPDT)
        W2p = np.ascontiguousarray(
            W2[e].reshape(F // (KC * P), KC, P, H // P, P).transpose(3, 0, 2, 1, 4)
        ).astype(_NPDT)
        in_maps.append({
            "xT": xT_t,
            "W1": W1p,
            "W2": W2p,
            "b1": np.ascontiguousarray(b1[e].reshape(F // P, P).T),
            "b2": np.ascontiguousarray(b2[e].reshape(H // P, P).T),
        })
    return in_maps


def kernel(x, Wg, W1, b1, W2, b2):
    x = np.asarray(x, dtype=np.float32)
    Wg = np.asarray(Wg, dtype=np.float32)
    W1 = np.asarray(W1, dtype=np.float32)
    b1 = np.asarray(b1, dtype=np.float32)
    W2 = np.asarray(W2, dtype=np.float32)
    b2 = np.asarray(b2, dtype=np.float32)

    tokens = x.reshape(T, H)
    tok_ids, gate_val = _route(tokens, Wg)
    in_maps = _make_in_maps(x, W1, b1, W2, b2, tok_ids)

    nc = _get_nc()
    res = run_bass_kernel_spmd(nc, in_maps, list(range(NCORES)))

    # combine: scatter expert outputs back, scaled by the gate value
    out = np.zeros((T, H), dtype=np.float32)
    for e in range(E):
        ids = tok_ids[e]
        yT = res.results[e]["out"]                   # [H, C]
        out[ids] = yT[:, :len(ids)].T * gate_val[ids, None]
    return out.reshape(B, S, H)


# revision 4
# speedup vs baseline: 1.0878x; 1.0153x over previous
"""MoE (top-1 routing, capacity-dropped) forward on 8 Trainium2 NeuronCores.

bf16 variant of the expert-parallel kernel: weights + activations are cast to
bfloat16 on the host (PE runs bf16 at the same 1-elem/cell/cycle rate as
float32r, so matmul cycle count is unchanged) which
  - halves the weight/activation HBM traffic (33.5 MB -> 16.8 MB per core),
  - enables Fast Weight Load (FWL is disabled for FP32 weights),
  - halves SBUF pressure (h1 goes 16 MB -> 8 MB),
  - draws less PE/SBUF/DMA power (fp32r full-rate is the worst-case power
    mode; sustained high draw downclocks PE 2.4 -> ~2.0 GHz via P0).
Accumulation stays fp32 in PSUM; biases and the final output stay fp32.

Strategy (expert-parallel, per the sharding hint):
  - Host computes top-1 gating + capacity dropping and dispatches token rows
    to their expert (tokens sharded along E, one expert per core; W1/b1/W2/b2
    sharded along E; gate Wg applied once on the full token set).
  - Each core: yT = W2[e].T @ relu(W1[e].T @ xT + b1[e]) + b2[e] on its
    C=1024 dispatched tokens in [feature, token] layout.
  - Host combine: scatter expert outputs back scaled by the gate value.

Hardcoded shapes: x [4, 2048, 1024], Wg [1024, 8], W1 [8, 1024, 4096],
b1 [8, 4096], W2 [8, 4096, 1024], b2 [8, 1024].
"""

import os
import numpy as np
from contextlib import ExitStack

try:
    from antenv.axon_hooks import get_axon_ntff_profile_hook  # noqa: F401
except Exception:
    os.environ.setdefault("BASS_NEVER_TRACE", "1")

import ml_dtypes
from concourse import bacc, mybir, tile
from concourse.bass_utils import run_bass_kernel_spmd

B, S, H, F, E = 4, 2048, 1024, 4096, 8
T = B * S                  # 8192 tokens
C = -(-T // E)             # 1024 capacity per expert
P = 128                    # SBUF partitions
NB = 512                   # matmul moving free-dim block (one PSUM bank, fp32)
NCORES = 8

_DT = mybir.dt.bfloat16
_NPDT = ml_dtypes.bfloat16


def _build_expert_ffn(repeat: int = 1):
    """Per-core program: yT = W2.T @ relu(W1.T @ xT + b1) + b2, all [feat, tok].

    repeat>1 wraps the whole body in a hardware For loop — used only by the
    timing harness to measure steady-state per-iteration HW time via the
    wall-clock slope between repeat counts."""
    nc = bacc.Bacc("TRN2", target_bir_lowering=False, debug=False,
                   num_devices=NCORES)
    KC = 8  # k-tiles per streamed W2 chunk (matches the W1 slab size)
    # Weights arrive pre-tiled from the host (see kernel()) so every slab DMA
    # is a fully contiguous read per partition:
    # W1p[m, p, k, f] = W1[k*P+p, m*P+f] and
    # W2p[mh, kc, p, k, f] = W2[kc*KC*P + k*P + p, mh*P + f].
    NBLK = C // NB
    # x arrives host-pretiled as (n, k)-chunks, each a fully-contiguous
    # [P, NB] slab (1 KB/partition), so the first matmul waits on one 128 KB
    # chunk instead of the whole 2 MB transfer:
    # xT[n, k, p, c] = x_orig[k*P+p, n*NB+c].
    xT = nc.declare_dram_parameter(
        "xT", [NBLK, H // P, P, NB], _DT, isOutput=False)
    W1 = nc.declare_dram_parameter("W1", [F // P, P, H // P, P], _DT, isOutput=False)
    W2 = nc.declare_dram_parameter(
        "W2", [H // P, F // (KC * P), P, KC, P], _DT, isOutput=False)
    b1 = nc.declare_dram_parameter("b1", [P, F // P], mybir.dt.float32,
                                   isOutput=False)  # host-pretiled [p, m]
    b2 = nc.declare_dram_parameter("b2", [P, H // P], mybir.dt.float32,
                                   isOutput=False)
    out = nc.declare_dram_parameter("out", [H, C], mybir.dt.float32, isOutput=True)

    KH = H // P    # 8  k-tiles over H (mm1 contraction)
    KF = F // P    # 32 k-tiles over F (mm2 contraction)
    MF = F // P    # 32 m-tiles over F (mm1 output partitions)
    MH = H // P    # 8  m-tiles over H (mm2 output partitions)

    with tile.TileContext(nc) as tc, ExitStack() as ctx:
        xpool = ctx.enter_context(tc.tile_pool(name="xpool", bufs=1))
        h1pool = ctx.enter_context(tc.tile_pool(name="h1pool", bufs=1))
        cpool = ctx.enter_context(tc.tile_pool(name="cpool", bufs=1))
        wpool = ctx.enter_context(tc.tile_pool(name="wpool", bufs=6))
        ypool = ctx.enter_context(tc.tile_pool(name="ypool", bufs=2))
        psum = ctx.enter_context(tc.tile_pool(name="psum", bufs=6, space="PSUM"))

        # HAM warm-up: the PE clock-gate passes 4/8 pulses (1.2 GHz) until
        # the PE has been busy ~3.4 us. The first real matmul can't start
        # until the first x chunk + W1 slab DMAs land (~3 us), so burn that
        # idle window on throwaway matmuls over a memset tile — the real
        # matmul stream then starts at the warm 2.4 GHz clock. Outside the
        # repeat loop: once per program, like the harness's single shot.
        warm_sb = cpool.tile([P, P], _DT, tag="warm_sb", name="warm_sb")
        nc.vector.memset(warm_sb[:], 0.0)
        wpsum = ctx.enter_context(tc.tile_pool(name="wpsum", bufs=1, space="PSUM"))
        warm_ps = wpsum.tile([P, P], mybir.dt.float32, tag="warm", name="warm_ps")
        for _ in range(16):
            nc.tensor.matmul(out=warm_ps[:], lhsT=warm_sb[:], rhs=warm_sb[:],
                             start=True, stop=True)

        loop_ctx = tc.For_i(0, repeat, 1) if repeat > 1 else None
        if loop_ctx is not None:
            loop_ctx.__enter__()

        # resident activations, chunked (n, k) so compute starts after the
        # first 128 KB chunk; k-major issue order matches the k-outer
        # consumption order of mm1 (each k-step needs chunk k of BOTH
        # n-blocks: 852 ns of matmul per 728 ns of chunk arrival)
        x_sb = [[xpool.tile([P, NB], _DT, tag=f"x{n}_{k}", name=f"x_sb{n}_{k}")
                 for k in range(KH)] for n in range(NBLK)]
        for k in range(KH):
            for n in range(NBLK):
                nc.sync.dma_start(out=x_sb[n][k][:], in_=xT[n, k])

        # biases arrive host-pretiled: b1[p, m] = b1_orig[m*P + p]
        b1_sb = cpool.tile([P, MF], mybir.dt.float32, name="b1_sb")
        nc.gpsimd.dma_start(out=b1_sb[:], in_=b1[:])
        b2_sb = cpool.tile([P, MH], mybir.dt.float32, name="b2_sb")
        nc.gpsimd.dma_start(out=b2_sb[:], in_=b2[:])

        h1_sb = h1pool.tile([P, KF, C], _DT, name="h1_sb")

        # mm1: h1[m*P+p, c] = relu(b1[m*P+p] + sum_h W1[h, m*P+p] * xT[h, c])
        # k-outer: each weight k-tile stays stationary for both n-blocks
        # (halves the Ldweights count) with the two n-banks accumulating
        # concurrently in separate PSUM banks
        for m in range(MF):
            w1s = wpool.tile([P, KH, P], _DT, tag="wslab", name="w1s")
            nc.scalar.dma_start(out=w1s[:], in_=W1[m])
            ps = [psum.tile([P, NB], mybir.dt.float32, tag="ps", name="ps")
                  for _ in range(NBLK)]
            for k in range(KH):
                for n in range(NBLK):
                    nc.tensor.matmul(
                        out=ps[n][:],
                        lhsT=w1s[:, k, :],
                        rhs=x_sb[n][k][:],
                        start=(k == 0),
                        stop=(k == KH - 1),
                    )
            for n in range(NBLK):
                nc.scalar.activation(
                    out=h1_sb[:, m, n * NB:(n + 1) * NB],
                    in_=ps[n][:],
                    func=mybir.ActivationFunctionType.Relu,
                    bias=b1_sb[:, m:m + 1],
                )

        # mm2: y[mh*P+p, c] = b2[mh*P+p] + sum_f W2[f, mh*P+p] * h1[f, c]
        for mh in range(MH):
            w2chunks = []
            for kc in range(KF // KC):
                w2s = wpool.tile([P, KC, P], _DT, tag="wslab", name="w2s")
                nc.scalar.dma_start(out=w2s[:], in_=W2[mh, kc])
                w2chunks.append(w2s)
            if mh < MH - 1:
                # k-outer (as in mm1): halves the Ldweights count
                ps2 = [psum.tile([P, NB], mybir.dt.float32, tag="ps", name="ps2")
                       for _ in range(NBLK)]
                for k in range(KF):
                    for n in range(NBLK):
                        nc.tensor.matmul(
                            out=ps2[n][:],
                            lhsT=w2chunks[k // KC][:, k % KC, :],
                            rhs=h1_sb[:, k, n * NB:(n + 1) * NB],
                            start=(k == 0),
                            stop=(k == KF - 1),
                        )
                for n in range(NBLK):
                    y_sb = ypool.tile([P, NB], mybir.dt.float32, tag="y",
                                      name="y_sb")
                    nc.vector.tensor_add(
                        out=y_sb[:],
                        in0=ps2[n][:],
                        in1=b2_sb[:, mh:mh + 1].to_broadcast([P, NB]),
                    )
                    nc.gpsimd.dma_start(
                        out=out[mh * P:(mh + 1) * P, n * NB:(n + 1) * NB],
                        in_=y_sb[:],
                    )
                continue
            # last mh stays n-outer so its n-blocks retire ~7 us apart, and
            # the very last block runs as two 256-col accumulation groups
            # (same total PE cycles) so its first store overlaps the second
            # group's matmuls — halves the exposed drain tail
            for n in range(NBLK):
                last = (n == NBLK - 1)
                SB = NB // 2 if last else NB
                for s in range(NB // SB):
                    c0 = n * NB + s * SB
                    ps2 = psum.tile([P, NB], mybir.dt.float32, tag="ps", name="ps2")
                    for k in range(KF):
                        nc.tensor.matmul(
                            out=ps2[:, :SB],
                            lhsT=w2chunks[k // KC][:, k % KC, :],
                            rhs=h1_sb[:, k, c0:c0 + SB],
                            start=(k == 0),
                            stop=(k == KF - 1),
                        )
                    y_sb = ypool.tile([P, NB], mybir.dt.float32, tag="y", name="y_sb")
                    nc.vector.tensor_add(
                        out=y_sb[:, :SB],
                        in0=ps2[:, :SB],
                        in1=b2_sb[:, mh:mh + 1].to_broadcast([P, SB]),
                    )
                    nc.gpsimd.dma_start(
                        out=out[mh * P:(mh + 1) * P, c0:c0 + SB], in_=y_sb[:, :SB]
                    )
        if loop_ctx is not None:
            loop_ctx.__exit__(None, None, None)
    nc.compile()
    return nc


_NC_CACHE = None


def _get_nc():
    global _NC_CACHE
    if _NC_CACHE is None:
        _NC_CACHE = _build_expert_ffn()
    return _NC_CACHE


def _route(tokens: np.ndarray, Wg: np.ndarray):
    """Top-1 gating with capacity C, matching deepspeed top1gating semantics.

    Runs on jax-CPU mirroring the reference ops 1:1 so the discrete routing
    decisions (argmax expert, cumsum slot order, capacity drops) are
    bit-identical to the jax reference — a numpy BLAS logits matmul could
    flip the argmax for tokens whose top-2 logits are ~1e-6 apart.

    Returns per-expert kept token ids (slot order) and per-token gate values
    (0 for dropped tokens)."""
    import jax
    import jax.numpy as jnp

    cpu = jax.devices("cpu")[0]
    with jax.default_device(cpu):
        tok = jnp.asarray(tokens)
        logits = tok @ jnp.asarray(Wg)                       # [T, E]
        gates = jax.nn.softmax(logits, axis=-1)
        idx = jnp.argmax(gates, axis=1)
        mask1 = jax.nn.one_hot(idx, E, dtype=gates.dtype)
        locations1 = jnp.cumsum(mask1, axis=0) - 1.0
        mask1 = mask1 * (locations1 < C).astype(gates.dtype)
        gates1 = jnp.sum(gates * mask1, axis=1)              # 0 if dropped

        mask_np = np.asarray(mask1)
        gate_val = np.asarray(gates1, dtype=np.float32)

    tok_ids = [np.nonzero(mask_np[:, e] > 0)[0] for e in range(E)]
    return tok_ids, gate_val


def _make_in_maps(x, W1, b1, W2, b2, tok_ids):
    """Shard tokens along the expert axis and pre-tile weights to the
    kernel's SBUF slab layout (contiguous per-partition reads), in bf16."""
    KC = 8
    tokens = x.reshape(T, H)
    in_maps = []
    NBLK = C // NB
    for e in range(E):
        xT_e = np.zeros((H, C), dtype=_NPDT)
        ids = tok_ids[e]
        xT_e[:, :len(ids)] = tokens[ids].astype(_NPDT).T
        # kernel SBUF chunk layout: xT[n, k, p, c] = xT_e[k*P+p, n*NB+c]
        xT_t = np.ascontiguousarray(
            xT_e.reshape(H // P, P, NBLK, NB).transpose(2, 0, 1, 3))
        W1p = np.ascontiguousarray(
            W1[e].reshape(H // P, P, F // P, P).transpose(2, 1, 0, 3)
        ).astype(_NPDT)
        W2p = np.ascontiguousarray(
            W2[e].reshape(F // (KC * P), KC, P, H // P, P).transpose(3, 0, 2, 1, 4)
        ).astype(_NPDT)
        in_maps.append({
            "xT": xT_t,
            "W1": W1p,
            "W2": W2p,
            "b1": np.ascontiguousarray(b1[e].reshape(F // P, P).T),
            "b2": np.ascontiguousarray(b2[e].reshape(H // P, P).T),
        })
    return in_maps


def kernel(x, Wg, W1, b1, W2, b2):
    x = np.asarray(x, dtype=np.float32)
    Wg = np.asarray(Wg, dtype=np.float32)
    W1 = np.asarray(W1, dtype=np.float32)
    b1 = np.asarray(b1, dtype=np.float32)
    W2 = np.asarray(W2, dtype=np.float32)
    b2 = np.asarray(b2, dtype=np.float32)

    tokens = x.reshape(T, H)
    tok_ids, gate_val = _route(tokens, Wg)
    in_maps = _make_in_maps(x, W1, b1, W2, b2, tok_ids)

    nc = _get_nc()
    res = run_bass_kernel_spmd(nc, in_maps, list(range(NCORES)))

    # combine: scatter expert outputs back, scaled by the gate value
    out = np.zeros((T, H), dtype=np.float32)
    for e in range(E):
        ids = tok_ids[e]
        yT = res.results[e]["out"]                   # [H, C]
        out[ids] = yT[:, :len(ids)].T * gate_val[ids, None]
    return out.reshape(B, S, H)


# revision 5
# speedup vs baseline: 1.1133x; 1.0234x over previous
"""MoE (top-1 routing, capacity-dropped) forward on 8 Trainium2 NeuronCores.

bf16 variant of the expert-parallel kernel: weights + activations are cast to
bfloat16 on the host (PE runs bf16 at the same 1-elem/cell/cycle rate as
float32r, so matmul cycle count is unchanged) which
  - halves the weight/activation HBM traffic (33.5 MB -> 16.8 MB per core),
  - enables Fast Weight Load (FWL is disabled for FP32 weights),
  - halves SBUF pressure (h1 goes 16 MB -> 8 MB),
  - draws less PE/SBUF/DMA power (fp32r full-rate is the worst-case power
    mode; sustained high draw downclocks PE 2.4 -> ~2.0 GHz via P0).
Accumulation stays fp32 in PSUM; biases and the final output stay fp32.

Schedule: x is host-pretiled into sixteen contiguous 128 KB (n,k) chunks
issued k-major so the k-outer mm1 (each weight k-tile stationary for both
512-col n-blocks, two PSUM banks accumulating concurrently) consumes chunks
no faster than they arrive; 16 warm-up matmuls over a memset tile burn the
PE's cold-clock (HAM) window during the initial DMA wait; the final output
block runs as two 256-col accumulation groups so the last store overlaps
the last matmul group. PE floor: 1024 matmuls x 512 cycles = 218.5 us at
the warm 2.4 GHz clock; cost-model timeline of this schedule is 232.4 us.

Strategy (expert-parallel, per the sharding hint):
  - Host computes top-1 gating + capacity dropping and dispatches token rows
    to their expert (tokens sharded along E, one expert per core; W1/b1/W2/b2
    sharded along E; gate Wg applied once on the full token set).
  - Each core: yT = W2[e].T @ relu(W1[e].T @ xT + b1[e]) + b2[e] on its
    C=1024 dispatched tokens in [feature, token] layout.
  - Host combine: scatter expert outputs back scaled by the gate value.

Hardcoded shapes: x [4, 2048, 1024], Wg [1024, 8], W1 [8, 1024, 4096],
b1 [8, 4096], W2 [8, 4096, 1024], b2 [8, 1024].
"""

import os
import numpy as np
from contextlib import ExitStack

try:
    from antenv.axon_hooks import get_axon_ntff_profile_hook  # noqa: F401
except Exception:
    os.environ.setdefault("BASS_NEVER_TRACE", "1")

import ml_dtypes
from concourse import bacc, mybir, tile
from concourse.bass_utils import run_bass_kernel_spmd

B, S, H, F, E = 4, 2048, 1024, 4096, 8
T = B * S                  # 8192 tokens
C = -(-T // E)             # 1024 capacity per expert
P = 128                    # SBUF partitions
NB = 512                   # matmul moving free-dim block (one PSUM bank, fp32)
NCORES = 8

_DT = mybir.dt.bfloat16
_NPDT = ml_dtypes.bfloat16


def _build_expert_ffn(repeat: int = 1):
    """Per-core program: yT = W2.T @ relu(W1.T @ xT + b1) + b2, all [feat, tok].

    repeat>1 wraps the whole body in a hardware For loop — used only by the
    timing harness to measure steady-state per-iteration HW time via the
    wall-clock slope between repeat counts."""
    nc = bacc.Bacc("TRN2", target_bir_lowering=False, debug=False,
                   num_devices=NCORES)
    KC = 8  # k-tiles per streamed W2 chunk (matches the W1 slab size)
    # Weights arrive pre-tiled from the host (see kernel()) so every slab DMA
    # is a fully contiguous read per partition:
    # W1p[m, p, k, f] = W1[k*P+p, m*P+f] and
    # W2p[mh, kc, p, k, f] = W2[kc*KC*P + k*P + p, mh*P + f].
    NBLK = C // NB
    # x arrives host-pretiled as (n, k)-chunks, each a fully-contiguous
    # [P, NB] slab (1 KB/partition), so the first matmul waits on one 128 KB
    # chunk instead of the whole 2 MB transfer:
    # xT[n, k, p, c] = x_orig[k*P+p, n*NB+c].
    xT = nc.declare_dram_parameter(
        "xT", [NBLK, H // P, P, NB], _DT, isOutput=False)
    W1 = nc.declare_dram_parameter("W1", [F // P, P, H // P, P], _DT, isOutput=False)
    W2 = nc.declare_dram_parameter(
        "W2", [H // P, F // (KC * P), P, KC, P], _DT, isOutput=False)
    b1 = nc.declare_dram_parameter("b1", [P, F // P], mybir.dt.float32,
                                   isOutput=False)  # host-pretiled [p, m]
    b2 = nc.declare_dram_parameter("b2", [P, H // P], mybir.dt.float32,
                                   isOutput=False)
    out = nc.declare_dram_parameter("out", [H, C], mybir.dt.float32, isOutput=True)

    KH = H // P    # 8  k-tiles over H (mm1 contraction)
    KF = F // P    # 32 k-tiles over F (mm2 contraction)
    MF = F // P    # 32 m-tiles over F (mm1 output partitions)
    MH = H // P    # 8  m-tiles over H (mm2 output partitions)

    with tile.TileContext(nc) as tc, ExitStack() as ctx:
        xpool = ctx.enter_context(tc.tile_pool(name="xpool", bufs=1))
        h1pool = ctx.enter_context(tc.tile_pool(name="h1pool", bufs=1))
        cpool = ctx.enter_context(tc.tile_pool(name="cpool", bufs=1))
        wpool = ctx.enter_context(tc.tile_pool(name="wpool", bufs=6))
        ypool = ctx.enter_context(tc.tile_pool(name="ypool", bufs=2))
        psum = ctx.enter_context(tc.tile_pool(name="psum", bufs=6, space="PSUM"))

        # HAM warm-up: the PE clock-gate passes 4/8 pulses (1.2 GHz) until
        # the PE has been busy ~3.4 us. The first real matmul can't start
        # until the first x chunk + W1 slab DMAs land (~3 us), so burn that
        # idle window on throwaway matmuls over a memset tile — the real
        # matmul stream then starts at the warm 2.4 GHz clock. Outside the
        # repeat loop: once per program, like the harness's single shot.
        warm_sb = cpool.tile([P, P], _DT, tag="warm_sb", name="warm_sb")
        nc.vector.memset(warm_sb[:], 0.0)
        wpsum = ctx.enter_context(tc.tile_pool(name="wpsum", bufs=1, space="PSUM"))
        warm_ps = wpsum.tile([P, P], mybir.dt.float32, tag="warm", name="warm_ps")
        for _ in range(16):
            nc.tensor.matmul(out=warm_ps[:], lhsT=warm_sb[:], rhs=warm_sb[:],
                             start=True, stop=True)

        loop_ctx = tc.For_i(0, repeat, 1) if repeat > 1 else None
        if loop_ctx is not None:
            loop_ctx.__enter__()

        # resident activations, chunked (n, k) so compute starts after the
        # first 128 KB chunk; k-major issue order matches the k-outer
        # consumption order of mm1 (each k-step needs chunk k of BOTH
        # n-blocks: 852 ns of matmul per 728 ns of chunk arrival)
        x_sb = [[xpool.tile([P, NB], _DT, tag=f"x{n}_{k}", name=f"x_sb{n}_{k}")
                 for k in range(KH)] for n in range(NBLK)]
        for k in range(KH):
            for n in range(NBLK):
                nc.sync.dma_start(out=x_sb[n][k][:], in_=xT[n, k])

        # biases arrive host-pretiled: b1[p, m] = b1_orig[m*P + p]
        b1_sb = cpool.tile([P, MF], mybir.dt.float32, name="b1_sb")
        nc.gpsimd.dma_start(out=b1_sb[:], in_=b1[:])
        b2_sb = cpool.tile([P, MH], mybir.dt.float32, name="b2_sb")
        nc.gpsimd.dma_start(out=b2_sb[:], in_=b2[:])

        h1_sb = h1pool.tile([P, KF, C], _DT, name="h1_sb")

        # mm1: h1[m*P+p, c] = relu(b1[m*P+p] + sum_h W1[h, m*P+p] * xT[h, c])
        # k-outer: each weight k-tile stays stationary for both n-blocks
        # (halves the Ldweights count) with the two n-banks accumulating
        # concurrently in separate PSUM banks
        for m in range(MF):
            w1s = wpool.tile([P, KH, P], _DT, tag="wslab", name="w1s")
            nc.scalar.dma_start(out=w1s[:], in_=W1[m])
            ps = [psum.tile([P, NB], mybir.dt.float32, tag="ps", name="ps")
                  for _ in range(NBLK)]
            for k in range(KH):
                for n in range(NBLK):
                    nc.tensor.matmul(
                        out=ps[n][:],
                        lhsT=w1s[:, k, :],
                        rhs=x_sb[n][k][:],
                        start=(k == 0),
                        stop=(k == KH - 1),
                    )
            for n in range(NBLK):
                nc.scalar.activation(
                    out=h1_sb[:, m, n * NB:(n + 1) * NB],
                    in_=ps[n][:],
                    func=mybir.ActivationFunctionType.Relu,
                    bias=b1_sb[:, m:m + 1],
                )

        # mm2: y[mh*P+p, c] = b2[mh*P+p] + sum_f W2[f, mh*P+p] * h1[f, c]
        for mh in range(MH):
            w2chunks = []
            for kc in range(KF // KC):
                w2s = wpool.tile([P, KC, P], _DT, tag="wslab", name="w2s")
                nc.scalar.dma_start(out=w2s[:], in_=W2[mh, kc])
                w2chunks.append(w2s)
            if mh < MH - 1:
                # k-outer (as in mm1): halves the Ldweights count
                ps2 = [psum.tile([P, NB], mybir.dt.float32, tag="ps", name="ps2")
                       for _ in range(NBLK)]
                for k in range(KF):
                    for n in range(NBLK):
                        nc.tensor.matmul(
                            out=ps2[n][:],
                            lhsT=w2chunks[k // KC][:, k % KC, :],
                            rhs=h1_sb[:, k, n * NB:(n + 1) * NB],
                            start=(k == 0),
                            stop=(k == KF - 1),
                        )
                for n in range(NBLK):
                    y_sb = ypool.tile([P, NB], mybir.dt.float32, tag="y",
                                      name="y_sb")
                    nc.vector.tensor_add(
                        out=y_sb[:],
                        in0=ps2[n][:],
                        in1=b2_sb[:, mh:mh + 1].to_broadcast([P, NB]),
                    )
                    nc.gpsimd.dma_start(
                        out=out[mh * P:(mh + 1) * P, n * NB:(n + 1) * NB],
                        in_=y_sb[:],
                    )
                continue
            # last mh stays n-outer so its n-blocks retire ~7 us apart, and
            # the very last block runs as two 256-col accumulation groups
            # (same total PE cycles) so its first store overlaps the second
            # group's matmuls — halves the exposed drain tail
            for n in range(NBLK):
                last = (n == NBLK - 1)
                SB = NB // 2 if last else NB
                for s in range(NB // SB):
                    c0 = n * NB + s * SB
                    ps2 = psum.tile([P, NB], mybir.dt.float32, tag="ps", name="ps2")
                    for k in range(KF):
                        nc.tensor.matmul(
                            out=ps2[:, :SB],
                            lhsT=w2chunks[k // KC][:, k % KC, :],
                            rhs=h1_sb[:, k, c0:c0 + SB],
                            start=(k == 0),
                            stop=(k == KF - 1),
                        )
                    y_sb = ypool.tile([P, NB], mybir.dt.float32, tag="y", name="y_sb")
                    nc.vector.tensor_add(
                        out=y_sb[:, :SB],
                        in0=ps2[:, :SB],
                        in1=b2_sb[:, mh:mh + 1].to_broadcast([P, SB]),
                    )
                    nc.gpsimd.dma_start(
                        out=out[mh * P:(mh + 1) * P, c0:c0 + SB], in_=y_sb[:, :SB]
                    )
        if loop_ctx is not None:
            loop_ctx.__exit__(None, None, None)
    nc.compile()
    return nc


_NC_CACHE = None


def _get_nc():
    global _NC_CACHE
    if _NC_CACHE is None:
        _NC_CACHE = _build_expert_ffn()
    return _NC_CACHE


def _route(tokens: np.ndarray, Wg: np.ndarray):
    """Top-1 gating with capacity C, matching deepspeed top1gating semantics.

    Runs on jax-CPU mirroring the reference ops 1:1 so the discrete routing
    decisions (argmax expert, cumsum slot order, capacity drops) are
    bit-identical to the jax reference — a numpy BLAS logits matmul could
    flip the argmax for tokens whose top-2 logits are ~1e-6 apart.

    Returns per-expert kept token ids (slot order) and per-token gate values
    (0 for dropped tokens)."""
    import jax
    import jax.numpy as jnp

    cpu = jax.devices("cpu")[0]
    with jax.default_device(cpu):
        tok = jnp.asarray(tokens)
        logits = tok @ jnp.asarray(Wg)                       # [T, E]
        gates = jax.nn.softmax(logits, axis=-1)
        idx = jnp.argmax(gates, axis=1)
        mask1 = jax.nn.one_hot(idx, E, dtype=gates.dtype)
        locations1 = jnp.cumsum(mask1, axis=0) - 1.0
        mask1 = mask1 * (locations1 < C).astype(gates.dtype)
        gates1 = jnp.sum(gates * mask1, axis=1)              # 0 if dropped

        mask_np = np.asarray(mask1)
        gate_val = np.asarray(gates1, dtype=np.float32)

    tok_ids = [np.nonzero(mask_np[:, e] > 0)[0] for e in range(E)]
    return tok_ids, gate_val


def _make_in_maps(x, W1, b1, W2, b2, tok_ids):
    """Shard tokens along the expert axis and pre-tile weights to the
    kernel's SBUF slab layout (contiguous per-partition reads), in bf16."""
    KC = 8
    tokens = x.reshape(T, H)
    in_maps = []
    NBLK = C // NB
    for e in range(E):
        xT_e = np.zeros((H, C), dtype=_NPDT)
        ids = tok_ids[e]
        xT_e[:, :len(ids)] = tokens[ids].astype(_NPDT).T
        # kernel SBUF chunk layout: xT[n, k, p, c] = xT_e[k*P+p, n*NB+c]
        xT_t = np.ascontiguousarray(
            xT_e.reshape(H // P, P, NBLK, NB).transpose(2, 0, 1, 3))
        W1p = np.ascontiguousarray(
            W1[e].reshape(H // P, P, F // P, P).transpose(2, 1, 0, 3)
        ).astype(_NPDT)
        W2p = np.ascontiguousarray(
            W2[e].reshape(F // (KC * P), KC, P, H // P, P).transpose(3, 0, 2, 1, 4)
        ).astype(_NPDT)
        in_maps.append({
            "xT": xT_t,
            "W1": W1p,
            "W2": W2p,
            "b1": np.ascontiguousarray(b1[e].reshape(F // P, P).T),
            "b2": np.ascontiguousarray(b2[e].reshape(H // P, P).T),
        })
    return in_maps


def kernel(x, Wg, W1, b1, W2, b2):
    x = np.asarray(x, dtype=np.float32)
    Wg = np.asarray(Wg, dtype=np.float32)
    W1 = np.asarray(W1, dtype=np.float32)
    b1 = np.asarray(b1, dtype=np.float32)
    W2 = np.asarray(W2, dtype=np.float32)
    b2 = np.asarray(b2, dtype=np.float32)

    tokens = x.reshape(T, H)
    tok_ids, gate_val = _route(tokens, Wg)
    in_maps = _make_in_maps(x, W1, b1, W2, b2, tok_ids)

    nc = _get_nc()
    res = run_bass_kernel_spmd(nc, in_maps, list(range(NCORES)))

    # combine: scatter expert outputs back, scaled by the gate value
    out = np.zeros((T, H), dtype=np.float32)
    for e in range(E):
        ids = tok_ids[e]
        yT = res.results[e]["out"]                   # [H, C]
        out[ids] = yT[:, :len(ids)].T * gate_val[ids, None]
    return out.reshape(B, S, H)
